# revision 71
# baseline (speedup 1.0000x reference)
"""GQA attention (B=2, S=2048, D=2048, 32 Q heads / 8 KV heads, HD=64) on 8 trn2 cores.

Sharding: tensor-parallel over heads. Core c gets Q heads [4c, 4c+4), KV head c.
Each core computes a full [B*S, D] partial of the output (its 4 heads through
o_proj); the host sums the 8 bf16 partials in f64. No collectives.

On-chip layout:
  - host passes hidden^T [D, B*S] (bf16) so every projection matmul has the
    contraction dim on partitions without any on-chip transpose;
  - K and V projections are fused into one matmul (wkv packs both weight
    blocks); V^T is then flipped to [keys, hd] with tiny PE transposes;
  - scores are computed transposed (scoresT[k, q] = K^T-stationary @ Q^T-moving)
    so the PV matmul consumes exp(scoresT) directly as the moving operand;
  - V is augmented with a ones column -> PV's PSUM row 64 accumulates the
    softmax denominators for free; normalization is applied to the small
    attnT [64, q] result (reciprocal + gpsimd partition_broadcast + DVE mul);
  - attnT [hd, q] is exactly the lhsT the o_proj matmul needs.

Schedule ("ride" software pipeline): the exp of head r's scores is the ACT
bottleneck, so while head r's scores stream, the PE interleaves the PV
matmuls of head r-1 (whose exp tiles all exist) plus filler work pulled
from a generator queue (remaining QKV projection chunks, then o_proj of
the previous head group). PSUM: 2x[128,1024] score tiles + 1x[65,1024] PV
accumulator + 2x[128,512] projection/o_proj tiles = exactly 8 banks.
"""

import functools
from collections import deque

import numpy as np
import ml_dtypes

import concourse.bacc as bacc
import concourse.bass as bass
import concourse.mybir as mybir
import concourse.tile as tile
from concourse.bass_utils import run_bass_kernel_spmd

B, S, D = 2, 2048, 2048
H, KVH, HD = 32, 8, 64
NCORES = 8
QH = H // NCORES            # 4 q heads per core
ST = B * S                  # 4096 flattened rows
QHD = QH * HD               # 256 (q hd dims per core)
SCALE = 1.0 / np.sqrt(HD)

BF16 = mybir.dt.bfloat16
F32 = mybir.dt.float32

DC = D // 128               # 16 contraction chunks
SC_N = ST // 512            # 8 s-chunks for projections
KB_N = S // 128             # 16 key blocks per batch
QHALF = 1024                # q columns per attention job


def build_program(trace_friendly: bool = False):
    nc = bacc.Bacc("TRN2", target_bir_lowering=False)
    ht = nc.dram_tensor("ht", [D, ST], BF16, kind="ExternalInput")
    wq = nc.dram_tensor("wq", [D, QHD], BF16, kind="ExternalInput")
    wk = nc.dram_tensor("wk", [D, HD], BF16, kind="ExternalInput")
    wv = nc.dram_tensor("wv", [D, HD], BF16, kind="ExternalInput")
    wo = nc.dram_tensor("wo", [QHD, D], BF16, kind="ExternalInput")
    out = nc.dram_tensor("out", [ST, D], BF16, kind="ExternalOutput")

    with tile.TileContext(nc) as tc:
        with (
            tc.tile_pool(name="singles", bufs=1) as singles,
            tc.tile_pool(name="hstream", bufs=2) as hstream,
            tc.tile_pool(name="expp", bufs=20) as expp,
            tc.tile_pool(name="attn", bufs=2) as attnp,
            tc.tile_pool(name="norm", bufs=2) as normp,
            tc.tile_pool(name="ostage", bufs=3) as ostage,
            tc.tile_pool(name="vtp", bufs=2) as vtp,
            tc.tile_pool(name="ps_sc", bufs=2, space="PSUM") as ps_sc,
            tc.tile_pool(name="ps_out", bufs=1, space="PSUM") as ps_out,
            tc.tile_pool(name="ps_op", bufs=2, space="PSUM") as ps_op,
        ):
            # ---- resident weights (one batched DMA per tensor) ----
            # wkv packs K | V columns so one matmul computes both projections:
            # out rows 0:64 = K^T chunk, rows 64:128 = V^T chunk.
            wq_sb = singles.tile([128, DC, QHD], BF16)
            wkv_sb = singles.tile([128, DC, 2 * HD], BF16)
            wo_sb = singles.tile([128, 2, D], BF16)
            # first h-chunk load goes out before the weights: the first Q
            # matmul needs both, and the h transfer is the long pole
            h_sb0 = hstream.tile([128, DC, 512], BF16, tag="h_sb",
                                 name="h_sb0")
            # split the first h-chunk and wq loads in quarters so the very
            # first Q matmuls (low dc) can start as soon as possible
            for q in range(4):
                r0, r1 = q * 512, (q + 1) * 512
                d0, d1 = q * (DC // 4), (q + 1) * (DC // 4)
                nc.sync.dma_start(
                    wq_sb[:, d0:d1],
                    wq[r0:r1, :].rearrange("(dc p) n -> p dc n", p=128))
                nc.sync.dma_start(
                    h_sb0[:, d0:d1],
                    ht[r0:r1, 0:512].rearrange("(dc p) n -> p dc n", p=128))
            nc.sync.dma_start(
                wkv_sb[:, :, 0:HD],
                wk[:, :].rearrange("(dc p) n -> p dc n", p=128))
            nc.sync.dma_start(
                wkv_sb[:, :, HD:2 * HD],
                wv[:, :].rearrange("(dc p) n -> p dc n", p=128))
            ident = singles.tile([64, 64], BF16)
            from concourse.masks import make_identity
            make_identity(nc, ident)

            # ---- resident activations (per batch) ----
            qt_sb = [[singles.tile([64, S], BF16, tag=f"qt{h}_{b}",
                                   name=f"qt{h}_{b}")
                      for b in range(B)] for h in range(QH)]
            kt_sb = [singles.tile([64, S], BF16, tag=f"kt{b}", name=f"kt{b}")
                     for b in range(B)]
            vaug_sb = [singles.tile([128, KB_N, HD + 1], BF16, tag=f"vaug{b}",
                                    name=f"vaug{b}")
                       for b in range(B)]
            for b in range(B):
                nc.vector.memset(vaug_sb[b][:, :, HD:HD + 1], 1.0)

            # ================= phase 1: QKV projections =================
            # b-major s-chunks so batch-0 attention can start early. The
            # last chunk is deferred into the ride stream (ride 0 has no PV
            # work yet, so the PE slack there absorbs it).
            def dma_h(sc):
                h_sb = hstream.tile([128, DC, 512], BF16, tag="h_sb",
                                    name=f"h_sb{sc}")
                nc.sync.dma_start(
                    h_sb[:],
                    ht[:, sc * 512:(sc + 1) * 512].rearrange(
                        "(dc p) n -> p dc n", p=128))
                return h_sb

            def kv_work(sc, h_sb):
                b = sc // (SC_N // B)
                scol = (sc % (SC_N // B)) * 512  # column offset within batch
                # K^T and V^T in one matmul (wkv stationary packs both)
                pkv = ps_op.tile([128, 512], F32, tag="po")
                for dc in range(DC):
                    nc.tensor.matmul(pkv, wkv_sb[:, dc], h_sb[:, dc],
                                     start=(dc == 0), stop=(dc == DC - 1))
                    yield
                nc.vector.tensor_copy(kt_sb[b][:, scol:scol + 512], pkv[0:64])
                vt = vtp.tile([64, 512], BF16)
                nc.vector.tensor_copy(vt, pkv[64:128])
                # transpose V^T [64, s] chunks -> vaug [s, 64] via PE
                for sb in range(4):
                    kb = (scol // 512) * 4 + sb
                    pvt = ps_op.tile([128, 512], F32, tag="po")
                    pvt_bf = pvt.bitcast(BF16)[:, 0:HD]
                    nc.tensor.transpose(
                        pvt_bf, vt[:, sb * 128:(sb + 1) * 128], ident)
                    yield
                    nc.vector.tensor_copy(vaug_sb[b][:, kb, 0:HD], pvt_bf)

            def q_work(sc, h_sb):
                b = sc // (SC_N // B)
                scol = (sc % (SC_N // B)) * 512
                # Q^T: two 128-row chunks of hd
                for m in range(2):
                    pq = ps_op.tile([128, 512], F32, tag="po")
                    for dc in range(DC):
                        nc.tensor.matmul(
                            pq,
                            wq_sb[:, dc, m * 128:(m + 1) * 128],
                            h_sb[:, dc],
                            start=(dc == 0), stop=(dc == DC - 1))
                        yield
                    for hs in range(2):
                        h = 2 * m + hs
                        nc.vector.tensor_copy(
                            qt_sb[h][b][:, scol:scol + 512],
                            pq[hs * 64:(hs + 1) * 64, :])

            def full_work(sc, dma_next=None):
                # body runs lazily at first next(): h_tiles[sc] exists by
                # then (created by the predecessor generator's dma_next)
                if dma_next is not None:
                    h_tiles[dma_next] = dma_h(dma_next)
                h_sb = h_tiles[sc]
                yield from kv_work(sc, h_sb)
                yield from q_work(sc, h_sb)

            # inline prefix: everything ride 0/1 depends on (kt/vaug of
            # batch 0 and the first q-half's qt); the rest streams through
            # the ride filler queue. Sections are round-robined so one
            # section's transposes hide in another's accumulation matmuls.
            def drive_rr(gens):
                gens = deque(gens)
                while gens:
                    g = gens.popleft()
                    try:
                        next(g)
                        next(g)
                        gens.append(g)
                    except StopIteration:
                        pass

            h_tiles = {0: h_sb0}
            for sc in (1, 2, 3):
                h_tiles[sc] = dma_h(sc)
            # o_proj weights only matter from the first o_proj ride (~40%
            # in): load them after the h-chunk stream
            nc.sync.dma_start(
                wo_sb[:], wo[:, :].rearrange("(hh p) n -> p hh n", p=128))
            # NOTE: at most TWO sections per round-robin — each section can
            # hold one "po" PSUM buffer mid-accumulation and the pool only
            # has two; a trio deadlocks the in-order PE queue.
            drive_rr([q_work(0, h_tiles[0])])
            drive_rr([kv_work(0, h_tiles[0]), q_work(1, h_tiles[1])])
            drive_rr([kv_work(1, h_tiles[1]), kv_work(2, h_tiles[2])])
            drive_rr([kv_work(3, h_tiles[3])])
            h_tiles[4] = dma_h(4)
            h_tiles[5] = dma_h(5)
            proj_gens = [
                q_work(2, h_tiles[2]),
                q_work(3, h_tiles[3]),
                full_work(4, dma_next=6),
                full_work(5, dma_next=7),
                full_work(6),
                full_work(7),
            ]

            # ============ phase 2: attention + o_proj ============
            # Ride-along software pipeline: during ride r the PE streams
            # scores+exp for head r while the PV matmuls of head r-1 (whose
            # exp tiles all exist) and o_proj matmuls of the previous head
            # GROUP interleave into the exp-wait bubbles. PV start is
            # delayed PV_DELAY iterations so the norm of head r-2 (same
            # PSUM accumulator, bufs=1) has drained.


            def oproj_gen(attn_t, b_, q0_, act_copies=False, qcs=None):
                for qc in qcs if qcs is not None else range(QHALF // 128):
                    osb = ostage.tile([128, D], BF16)
                    for nb in range(D // 512):
                        po = ps_op.tile([128, 512], F32, tag="po")
                        for hh in range(2):
                            nc.tensor.matmul(
                                po, attn_t[:, hh, qc * 128:(qc + 1) * 128],
                                wo_sb[:, hh, nb * 512:(nb + 1) * 512],
                                start=(hh == 0), stop=(hh == 1))
                            yield
                        if act_copies and nb % 2 == 0:
                            # tail has no exp work: ACT takes half the copies
                            nc.scalar.activation(
                                osb[:, nb * 512:(nb + 1) * 512], po,
                                mybir.ActivationFunctionType.Identity)
                        else:
                            nc.vector.tensor_copy(
                                osb[:, nb * 512:(nb + 1) * 512], po)
                    row = b_ * S + q0_ + qc * 128
                    if act_copies:
                        # tail: halve the stores so the final DMA (which
                        # gates the drain) starts and finishes sooner
                        nc.sync.dma_start(
                            out[row:row + 128, 0:D // 2], osb[:, 0:D // 2])
                        nc.sync.dma_start(
                            out[row:row + 128, D // 2:], osb[:, D // 2:])
                    else:
                        nc.sync.dma_start(out[row:row + 128, :], osb)

            ogen = deque(proj_gens)
            pending_ogen = deque()   # normed groups waiting for entry iter

            def fill(n):
                k = 0
                while k < n and ogen:
                    try:
                        next(ogen[0])
                        k += 1
                    except StopIteration:
                        ogen.popleft()
                return k

            rides = [(b, qh, h)
                     for b in range(B) for qh in range(S // QHALF)
                     for h in range(QH)]
            # PV of ride r-1 spans iters 2..12 of ride r; norm fires mid-ride
            # right after the last PV unit so by the NEXT ride's start the
            # outp accumulator (WAR) and the attn tile (RAW for o_proj) are
            # both clear.
            # per-iter PE feed (2 scores + pv + fills) tracks the ACT rate:
            # front iters 2sc+3pv = 1278ns, tail iters 2sc+3op = 1065ns vs
            # exp at 1038ns. PV ends iter 10 so the norm chain drains before
            # the NEXT ride's PV start=True hits the same accumulator.
            pv_sched = [4] * 8 + [0] * 8
            assert sum(pv_sched) == 2 * KB_N and len(pv_sched) == KB_N
            # keep the PE fed ~6 matmuls/iter: fillers top up whatever the
            # scores+PV stream leaves
            fill_sched = [max(0, 6 - 2 - npv) for npv in pv_sched]
            OPROJ_ENTRY_ITER = 13   # lets the norm mul finish before o_proj
            # the flush ride has no scores to pace against: PV all-out so the
            # last norm (and the last group's o_proj) starts ASAP
            pv_sched_flush = [16] * 2 + [0] * 14

            attn_by_group = {}

            def norm_head(pb, pqh, ph, outp, c0=0, c1=QHALF):
                gkey = (pb, pqh)
                if gkey not in attn_by_group:
                    attn_by_group[gkey] = attnp.tile(
                        [128, 2, QHALF], BF16, tag="attn",
                        name=f"attn_{pb}_{pqh}")
                attn_t = attn_by_group[gkey]
                recip = normp.tile([1, QHALF], F32, tag="recip")
                nc.vector.reciprocal(recip[:, c0:c1], outp[HD:HD + 1, c0:c1])
                bcast = normp.tile([64, QHALF], F32, tag="bcast")
                nc.gpsimd.partition_broadcast(
                    bcast[:, c0:c1], recip[:, c0:c1])
                nc.vector.tensor_mul(
                    attn_t[(ph % 2) * 64:(ph % 2) * 64 + 64, ph // 2, c0:c1],
                    outp[0:HD, c0:c1], bcast[:, c0:c1])
                if ph == QH - 1 and c1 == QHALF:
                    last = (pb, pqh) == (B - 1, S // QHALF - 1)
                    pending_ogen.append(
                        oproj_gen(attn_t, pb, pqh * QHALF, act_copies=last))
                    del attn_by_group[gkey]

            prev = prev_exp = None
            for r in range(len(rides) + 1):
                cur = rides[r] if r < len(rides) else None
                if prev is not None:
                    pb, pqh, ph = prev
                    outp = ps_out.tile([HD + 1, QHALF], F32)
                    if cur is None:
                        # flush: qq-major so each half's norm fires as soon
                        # as that half's accumulation completes
                        pv_units = [(qq, kb)
                                    for qq in range(2) for kb in range(KB_N)]
                    else:
                        pv_units = [(qq, kb)
                                    for kb in range(KB_N) for qq in range(2)]
                cur_exp = {}
                for it in range(KB_N):
                    if cur is not None:
                        cb, cqh, ch = cur
                        q0 = cqh * QHALF
                        scp = ps_sc.tile([128, QHALF], F32)
                        for qq in range(2):
                            nc.tensor.matmul(
                                scp[:, qq * 512:(qq + 1) * 512],
                                kt_sb[cb][:, it * 128:(it + 1) * 128],
                                qt_sb[ch][cb][:, q0 + qq * 512:q0 + (qq + 1) * 512],
                                start=True, stop=True)
                        expT = expp.tile([128, QHALF], BF16)
                        nc.scalar.activation(
                            expT, scp, mybir.ActivationFunctionType.Exp,
                            scale=SCALE)
                        cur_exp[it] = expT
                    sched = pv_sched if cur is not None else pv_sched_flush
                    npv = sched[it] if prev is not None else 0
                    for _ in range(npv):
                        qq, kb = pv_units.pop(0)
                        nc.tensor.matmul(
                            outp[:, qq * 512:(qq + 1) * 512],
                            vaug_sb[pb][:, kb],
                            prev_exp[kb][:, qq * 512:(qq + 1) * 512],
                            start=(kb == 0), stop=(kb == KB_N - 1),
                            skip_group_check=True)
                        if cur is None and len(pv_units) == KB_N:
                            norm_head(pb, pqh, ph, outp, 0, 512)
                    if prev is not None and not pv_units and outp is not None:
                        if cur is None:
                            norm_head(pb, pqh, ph, outp, 512, QHALF)
                        else:
                            norm_head(pb, pqh, ph, outp)
                        outp = None
                    if it >= OPROJ_ENTRY_ITER or cur is None:
                        while pending_ogen:
                            ogen.append(pending_ogen.popleft())
                    fill(fill_sched[it] if cur is not None else 3)
                prev, prev_exp = cur, cur_exp
            while pending_ogen:
                ogen.append(pending_ogen.popleft())
            while ogen:
                fill(1 << 20)
    nc.compile()
    return nc


@functools.lru_cache(maxsize=1)
def _get_program():
    return build_program()


def kernel(hidden_states, Wq, Wk, Wv, Wo):
    hidden_states = np.asarray(hidden_states)
    Wq, Wk, Wv, Wo = (np.asarray(x) for x in (Wq, Wk, Wv, Wo))
    bf = ml_dtypes.bfloat16

    htT = np.ascontiguousarray(
        hidden_states.reshape(ST, D).T.astype(bf))          # [D, B*S]
    in_maps = []
    for c in range(NCORES):
        in_maps.append({
            "ht": htT,
            "wq": np.ascontiguousarray(Wq[:, c * QHD:(c + 1) * QHD].astype(bf)),
            "wk": np.ascontiguousarray(Wk[:, c * HD:(c + 1) * HD].astype(bf)),
            "wv": np.ascontiguousarray(Wv[:, c * HD:(c + 1) * HD].astype(bf)),
            "wo": np.ascontiguousarray(Wo[c * QHD:(c + 1) * QHD, :].astype(bf)),
        })

    nc = _get_program()
    res = run_bass_kernel_spmd(nc, in_maps, core_ids=list(range(NCORES)))
    total = res.results[0]["out"].astype(np.float64)
    for c in range(1, NCORES):
        total += res.results[c]["out"].astype(np.float64)
    return total.reshape(B, S, D).astype(np.float32)



# revision 81
# speedup vs baseline: 1.0422x; 1.0422x over previous
"""GQA attention (B=2, S=2048, D=2048, 32 Q heads / 8 KV heads, HD=64) on 8 trn2 cores.

Sharding: tensor-parallel over heads. Core c gets Q heads [4c, 4c+4), KV head c.
Each core computes a full [B*S, D] partial of the output (its 4 heads through
o_proj); the host sums the 8 bf16 partials in f64. No collectives.

On-chip layout:
  - host passes hidden^T [D, B*S] (bf16) so every projection matmul has the
    contraction dim on partitions without any on-chip transpose;
  - K and V projections are fused into one matmul (wkv packs both weight
    blocks); V^T is then flipped to [keys, hd] with tiny PE transposes;
  - scores are computed transposed (scoresT[k, q] = K^T-stationary @ Q^T-moving)
    so the PV matmul consumes exp(scoresT) directly as the moving operand;
  - V is augmented with a ones column -> PV's PSUM row 64 accumulates the
    softmax denominators for free; normalization is applied to the small
    attnT [64, q] result (reciprocal + gpsimd partition_broadcast + DVE mul);
  - attnT [hd, q] is exactly the lhsT the o_proj matmul needs.

Schedule ("ride" software pipeline): the exp of head r's scores is the ACT
bottleneck, so while head r's scores stream, the PE interleaves the PV
matmuls of head r-1 (whose exp tiles all exist) plus filler work pulled
from a generator queue (remaining QKV projection chunks, then o_proj of
the previous head group). PSUM: 2x[128,1024] score tiles + 1x[65,1024] PV
accumulator + 2x[128,512] projection/o_proj tiles = exactly 8 banks.
"""

import functools
from collections import deque

import numpy as np
import ml_dtypes

import concourse.bacc as bacc
import concourse.bass as bass
import concourse.mybir as mybir
import concourse.tile as tile
from concourse.bass_utils import run_bass_kernel_spmd

B, S, D = 2, 2048, 2048
H, KVH, HD = 32, 8, 64
NCORES = 8
QH = H // NCORES            # 4 q heads per core
ST = B * S                  # 4096 flattened rows
QHD = QH * HD               # 256 (q hd dims per core)
SCALE = 1.0 / np.sqrt(HD)

BF16 = mybir.dt.bfloat16
F32 = mybir.dt.float32

DC = D // 128               # 16 contraction chunks
SC_N = ST // 512            # 8 s-chunks for projections
KB_N = S // 128             # 16 key blocks per batch
QHALF = 1024                # q columns per attention job


def build_program(trace_friendly: bool = False):
    nc = bacc.Bacc("TRN2", target_bir_lowering=False)
    ht = nc.dram_tensor("ht", [D, ST], BF16, kind="ExternalInput")
    wq = nc.dram_tensor("wq", [D, QHD], BF16, kind="ExternalInput")
    wk = nc.dram_tensor("wk", [D, HD], BF16, kind="ExternalInput")
    wv = nc.dram_tensor("wv", [D, HD], BF16, kind="ExternalInput")
    wo = nc.dram_tensor("wo", [QHD, D], BF16, kind="ExternalInput")
    out = nc.dram_tensor("out", [ST, D], BF16, kind="ExternalOutput")

    with tile.TileContext(nc) as tc:
        with (
            tc.tile_pool(name="singles", bufs=1) as singles,
            tc.tile_pool(name="hstream", bufs=2) as hstream,
            tc.tile_pool(name="expp", bufs=26) as expp,
            tc.tile_pool(name="attn", bufs=2) as attnp,
            tc.tile_pool(name="norm", bufs=2) as normp,
            tc.tile_pool(name="ostage", bufs=4) as ostage,
            tc.tile_pool(name="vtp", bufs=2) as vtp,
            tc.tile_pool(name="ps_sc", bufs=2, space="PSUM") as ps_sc,
            tc.tile_pool(name="ps_out", bufs=1, space="PSUM") as ps_out,
            tc.tile_pool(name="ps_op", bufs=2, space="PSUM") as ps_op,
        ):
            # ---- resident weights (one batched DMA per tensor) ----
            # wkv packs K | V columns so one matmul computes both projections:
            # out rows 0:64 = K^T chunk, rows 64:128 = V^T chunk.
            wq_sb = singles.tile([128, DC, QHD], BF16)
            wkv_sb = singles.tile([128, DC, 2 * HD], BF16)
            wo_sb = singles.tile([128, 2, D], BF16)
            # first h-chunk load goes out before the weights: the first Q
            # matmul needs both, and the h transfer is the long pole
            h_sb0 = hstream.tile([128, DC, 512], BF16, tag="h_sb",
                                 name="h_sb0")
            # split the first h-chunk and wq loads so the very first Q
            # matmuls (low dc) can start as soon as possible: eighth-sized
            # leading pieces, then quarters
            splits = [(0, 4), (4, 8), (8, 12), (12, 16)]
            for d0, d1 in splits:
                r0, r1 = d0 * 128, d1 * 128
                nc.sync.dma_start(
                    wq_sb[:, d0:d1],
                    wq[r0:r1, :].rearrange("(dc p) n -> p dc n", p=128))
                nc.sync.dma_start(
                    h_sb0[:, d0:d1],
                    ht[r0:r1, 0:512].rearrange("(dc p) n -> p dc n", p=128))
            nc.sync.dma_start(
                wkv_sb[:, :, 0:HD],
                wk[:, :].rearrange("(dc p) n -> p dc n", p=128))
            nc.sync.dma_start(
                wkv_sb[:, :, HD:2 * HD],
                wv[:, :].rearrange("(dc p) n -> p dc n", p=128))
            ident = singles.tile([64, 64], BF16)
            from concourse.masks import make_identity
            make_identity(nc, ident)

            # ---- resident activations (per batch) ----
            qt_sb = [[singles.tile([64, S], BF16, tag=f"qt{h}_{b}",
                                   name=f"qt{h}_{b}")
                      for b in range(B)] for h in range(QH)]
            kt_sb = [singles.tile([64, S], BF16, tag=f"kt{b}", name=f"kt{b}")
                     for b in range(B)]
            vaug_sb = [singles.tile([128, KB_N, HD + 1], BF16, tag=f"vaug{b}",
                                    name=f"vaug{b}")
                       for b in range(B)]
            for b in range(B):
                nc.vector.memset(vaug_sb[b][:, :, HD:HD + 1], 1.0)

            # ================= phase 1: QKV projections =================
            # b-major s-chunks so batch-0 attention can start early. The
            # last chunk is deferred into the ride stream (ride 0 has no PV
            # work yet, so the PE slack there absorbs it).
            def dma_h(sc):
                h_sb = hstream.tile([128, DC, 512], BF16, tag="h_sb",
                                    name=f"h_sb{sc}")
                nc.sync.dma_start(
                    h_sb[:],
                    ht[:, sc * 512:(sc + 1) * 512].rearrange(
                        "(dc p) n -> p dc n", p=128))
                return h_sb

            def kv_work(sc, h_sb):
                b = sc // (SC_N // B)
                scol = (sc % (SC_N // B)) * 512  # column offset within batch
                # K^T and V^T in one matmul (wkv stationary packs both)
                pkv = ps_op.tile([128, 512], F32, tag="po")
                for dc in range(DC):
                    nc.tensor.matmul(pkv, wkv_sb[:, dc], h_sb[:, dc],
                                     start=(dc == 0), stop=(dc == DC - 1))
                    yield
                nc.vector.tensor_copy(kt_sb[b][:, scol:scol + 512], pkv[0:64])
                vt = vtp.tile([64, 512], BF16)
                nc.vector.tensor_copy(vt, pkv[64:128])
                # transpose V^T [64, s] chunks -> vaug [s, 64] via PE
                for sb in range(4):
                    kb = (scol // 512) * 4 + sb
                    pvt = ps_op.tile([128, 512], F32, tag="po")
                    pvt_bf = pvt.bitcast(BF16)[:, 0:HD]
                    nc.tensor.transpose(
                        pvt_bf, vt[:, sb * 128:(sb + 1) * 128], ident)
                    yield
                    nc.vector.tensor_copy(vaug_sb[b][:, kb, 0:HD], pvt_bf)

            def q_work(sc, h_sb):
                b = sc // (SC_N // B)
                scol = (sc % (SC_N // B)) * 512
                # Q^T: two 128-row chunks of hd
                for m in range(2):
                    pq = ps_op.tile([128, 512], F32, tag="po")
                    for dc in range(DC):
                        nc.tensor.matmul(
                            pq,
                            wq_sb[:, dc, m * 128:(m + 1) * 128],
                            h_sb[:, dc],
                            start=(dc == 0), stop=(dc == DC - 1))
                        yield
                    for hs in range(2):
                        h = 2 * m + hs
                        nc.vector.tensor_copy(
                            qt_sb[h][b][:, scol:scol + 512],
                            pq[hs * 64:(hs + 1) * 64, :])

            def full_work(sc, dma_next=None):
                # body runs lazily at first next(): h_tiles[sc] exists by
                # then (created by the predecessor generator's dma_next)
                if dma_next is not None:
                    h_tiles[dma_next] = dma_h(dma_next)
                h_sb = h_tiles[sc]
                yield from kv_work(sc, h_sb)
                yield from q_work(sc, h_sb)

            # inline prefix: everything ride 0/1 depends on (kt/vaug of
            # batch 0 and the first q-half's qt); the rest streams through
            # the ride filler queue. Sections are round-robined so one
            # section's transposes hide in another's accumulation matmuls.
            def drive_rr(gens):
                gens = deque(gens)
                while gens:
                    g = gens.popleft()
                    try:
                        next(g)
                        next(g)
                        gens.append(g)
                    except StopIteration:
                        pass

            h_tiles = {0: h_sb0}
            for sc in (1, 2, 3):
                h_tiles[sc] = dma_h(sc)
            # o_proj weights only matter from the first o_proj ride (~40%
            # in): load them after the h-chunk stream
            nc.sync.dma_start(
                wo_sb[:], wo[:, :].rearrange("(hh p) n -> p hh n", p=128))
            # NOTE: at most TWO sections per round-robin — each section can
            # hold one "po" PSUM buffer mid-accumulation and the pool only
            # has two; a trio deadlocks the in-order PE queue.
            drive_rr([q_work(0, h_tiles[0])])
            drive_rr([kv_work(0, h_tiles[0]), q_work(1, h_tiles[1])])
            drive_rr([kv_work(1, h_tiles[1]), kv_work(2, h_tiles[2])])
            drive_rr([kv_work(3, h_tiles[3])])
            h_tiles[4] = dma_h(4)
            h_tiles[5] = dma_h(5)
            proj_gens = [
                q_work(2, h_tiles[2]),
                q_work(3, h_tiles[3]),
                full_work(4, dma_next=6),
                full_work(5, dma_next=7),
                full_work(6),
                full_work(7),
            ]

            # ============ phase 2: attention + o_proj ============
            # Ride-along software pipeline: during ride r the PE streams
            # scores+exp for head r while the PV matmuls of head r-1 (whose
            # exp tiles all exist) and o_proj matmuls of the previous head
            # GROUP interleave into the exp-wait bubbles. PV start is
            # delayed PV_DELAY iterations so the norm of head r-2 (same
            # PSUM accumulator, bufs=1) has drained.


            def oproj_gen(attn_t, b_, q0_, act_copies=False, qcs=None):
                for qc in qcs if qcs is not None else range(QHALF // 128):
                    osb = ostage.tile([128, D], BF16)
                    for nb in range(D // 512):
                        if act_copies and (qc * 4 + nb) % 2 == 0:
                            # tail: scores are finished, so their PSUM banks
                            # double the po rotation depth (copy latency no
                            # longer gates the matmul stream)
                            po = ps_sc.tile([128, 512], F32, tag="scp",
                                            name=f"po_t{qc}_{nb}")
                        else:
                            po = ps_op.tile([128, 512], F32, tag="po")
                        for hh in range(2):
                            nc.tensor.matmul(
                                po, attn_t[:, hh, qc * 128:(qc + 1) * 128],
                                wo_sb[:, hh, nb * 512:(nb + 1) * 512],
                                start=(hh == 0), stop=(hh == 1))
                            yield
                        if act_copies and nb % 2 == 0:
                            # tail has no exp work: ACT takes half the copies
                            nc.scalar.activation(
                                osb[:, nb * 512:(nb + 1) * 512], po,
                                mybir.ActivationFunctionType.Identity)
                        else:
                            nc.vector.tensor_copy(
                                osb[:, nb * 512:(nb + 1) * 512], po)
                    row = b_ * S + q0_ + qc * 128
                    if act_copies:
                        # tail: split the stores so the final DMA (which
                        # gates the drain) starts and finishes sooner
                        nsplit = 2
                        w = D // nsplit
                        for s in range(nsplit):
                            nc.sync.dma_start(
                                out[row:row + 128, s * w:(s + 1) * w],
                                osb[:, s * w:(s + 1) * w])
                    else:
                        nc.sync.dma_start(out[row:row + 128, :], osb)

            ogen = deque(proj_gens)
            pending_ogen = deque()   # normed groups waiting for entry iter

            def fill(n):
                k = 0
                while k < n and ogen:
                    try:
                        next(ogen[0])
                        k += 1
                    except StopIteration:
                        ogen.popleft()
                return k

            rides = [(b, qh, h)
                     for b in range(B) for qh in range(S // QHALF)
                     for h in range(QH)]
            # PV of ride r-1 spans iters 2..12 of ride r; norm fires mid-ride
            # right after the last PV unit so by the NEXT ride's start the
            # outp accumulator (WAR) and the attn tile (RAW for o_proj) are
            # both clear.
            # per-iter PE feed (2 scores + pv + fills) tracks the ACT rate:
            # front iters 2sc+3pv = 1278ns, tail iters 2sc+3op = 1065ns vs
            # exp at 1038ns. PV ends iter 10 so the norm chain drains before
            # the NEXT ride's PV start=True hits the same accumulator.
            pv_sched = [4] * 8 + [0] * 8
            assert sum(pv_sched) == 2 * KB_N and len(pv_sched) == KB_N
            # keep the PE fed ~6 matmuls/iter: fillers top up whatever the
            # scores+PV stream leaves
            fill_sched = [max(0, 6 - 2 - npv) for npv in pv_sched]
            OPROJ_ENTRY_ITER = 13   # lets the norm mul finish before o_proj
            # the flush ride has no scores to pace against: PV all-out so the
            # last norm (and the last group's o_proj) starts ASAP
            pv_sched_flush = [16] * 2 + [0] * 14
            pv_sched_last = [8, 8, 4, 4, 4, 4] + [0] * 10

            attn_by_group = {}

            def norm_head(pb, pqh, ph, outp, c0=0, c1=QHALF):
                gkey = (pb, pqh)
                if gkey not in attn_by_group:
                    attn_by_group[gkey] = attnp.tile(
                        [128, 2, QHALF], BF16, tag="attn",
                        name=f"attn_{pb}_{pqh}")
                attn_t = attn_by_group[gkey]
                # one f32 copy releases the PSUM accumulator ~3us earlier
                # than letting the recip->bcast->mul chain read it directly
                nsb = normp.tile([HD + 1, QHALF], F32, tag="nsb")
                nc.vector.tensor_copy(nsb[:, c0:c1], outp[:, c0:c1])
                recip = normp.tile([1, QHALF], F32, tag="recip")
                nc.vector.reciprocal(recip[:, c0:c1], nsb[HD:HD + 1, c0:c1])
                bcast = normp.tile([64, QHALF], F32, tag="bcast")
                nc.gpsimd.partition_broadcast(
                    bcast[:, c0:c1], recip[:, c0:c1])
                nc.vector.tensor_mul(
                    attn_t[(ph % 2) * 64:(ph % 2) * 64 + 64, ph // 2, c0:c1],
                    nsb[0:HD, c0:c1], bcast[:, c0:c1])
                if ph == QH - 1 and c1 == QHALF:
                    last = (pb, pqh) == (B - 1, S // QHALF - 1)
                    pending_ogen.append(
                        oproj_gen(attn_t, pb, pqh * QHALF, act_copies=last))
                    del attn_by_group[gkey]

            prev = prev_exp = None
            for r in range(len(rides) + 1):
                cur = rides[r] if r < len(rides) else None
                if prev is not None:
                    pb, pqh, ph = prev
                    outp = ps_out.tile([HD + 1, QHALF], F32)
                    if cur is None:
                        # flush: qq-major so each half's norm fires as soon
                        # as that half's accumulation completes
                        pv_units = [(qq, kb)
                                    for qq in range(2) for kb in range(KB_N)]
                    else:
                        pv_units = [(qq, kb)
                                    for kb in range(KB_N) for qq in range(2)]
                cur_exp = {}
                for it in range(KB_N):
                    if cur is not None:
                        cb, cqh, ch = cur
                        q0 = cqh * QHALF
                        scp = ps_sc.tile([128, QHALF], F32)
                        for qq in range(2):
                            nc.tensor.matmul(
                                scp[:, qq * 512:(qq + 1) * 512],
                                kt_sb[cb][:, it * 128:(it + 1) * 128],
                                qt_sb[ch][cb][:, q0 + qq * 512:q0 + (qq + 1) * 512],
                                start=True, stop=True)
                        expT = expp.tile([128, QHALF], BF16)
                        nc.scalar.activation(
                            expT, scp, mybir.ActivationFunctionType.Exp,
                            scale=SCALE)
                        cur_exp[it] = expT
                    sched = pv_sched if cur is not None else pv_sched_flush
                    npv = sched[it] if prev is not None else 0
                    for _ in range(npv):
                        qq, kb = pv_units.pop(0)
                        nc.tensor.matmul(
                            outp[:, qq * 512:(qq + 1) * 512],
                            vaug_sb[pb][:, kb],
                            prev_exp[kb][:, qq * 512:(qq + 1) * 512],
                            start=(kb == 0), stop=(kb == KB_N - 1),
                            skip_group_check=True)
                        if cur is None and len(pv_units) == KB_N:
                            norm_head(pb, pqh, ph, outp, 0, 512)
                    if prev is not None and not pv_units and outp is not None:
                        if cur is None:
                            norm_head(pb, pqh, ph, outp, 512, QHALF)
                        else:
                            norm_head(pb, pqh, ph, outp)
                        outp = None
                    if it >= OPROJ_ENTRY_ITER or cur is None:
                        while pending_ogen:
                            ogen.append(pending_ogen.popleft())
                    fill(fill_sched[it] if cur is not None else 3)
                prev, prev_exp = cur, cur_exp
            while pending_ogen:
                ogen.append(pending_ogen.popleft())
            while ogen:
                fill(1 << 20)
    nc.compile()
    return nc


@functools.lru_cache(maxsize=1)
def _get_program():
    return build_program()


def kernel(hidden_states, Wq, Wk, Wv, Wo):
    hidden_states = np.asarray(hidden_states)
    Wq, Wk, Wv, Wo = (np.asarray(x) for x in (Wq, Wk, Wv, Wo))
    bf = ml_dtypes.bfloat16

    htT = np.ascontiguousarray(
        hidden_states.reshape(ST, D).T.astype(bf))          # [D, B*S]
    in_maps = []
    for c in range(NCORES):
        in_maps.append({
            "ht": htT,
            "wq": np.ascontiguousarray(Wq[:, c * QHD:(c + 1) * QHD].astype(bf)),
            "wk": np.ascontiguousarray(Wk[:, c * HD:(c + 1) * HD].astype(bf)),
            "wv": np.ascontiguousarray(Wv[:, c * HD:(c + 1) * HD].astype(bf)),
            "wo": np.ascontiguousarray(Wo[c * QHD:(c + 1) * QHD, :].astype(bf)),
        })

    nc = _get_program()
    res = run_bass_kernel_spmd(nc, in_maps, core_ids=list(range(NCORES)))
    total = res.results[0]["out"].astype(np.float64)
    for c in range(1, NCORES):
        total += res.results[c]["out"].astype(np.float64)
    return total.reshape(B, S, D).astype(np.float32)



# revision 84
# speedup vs baseline: 1.0462x; 1.0038x over previous
"""GQA attention (B=2, S=2048, D=2048, 32 Q heads / 8 KV heads, HD=64) on 8 trn2 cores.

Sharding: tensor-parallel over heads. Core c gets Q heads [4c, 4c+4), KV head c.
Each core computes a full [B*S, D] partial of the output (its 4 heads through
o_proj); the host sums the 8 bf16 partials in f64. No collectives.

On-chip layout:
  - host passes hidden^T [D, B*S] (bf16) so every projection matmul has the
    contraction dim on partitions without any on-chip transpose;
  - K and V projections are fused into one matmul (wkv packs both weight
    blocks); V^T is then flipped to [keys, hd] with tiny PE transposes;
  - scores are computed transposed (scoresT[k, q] = K^T-stationary @ Q^T-moving)
    so the PV matmul consumes exp(scoresT) directly as the moving operand;
  - V is augmented with a ones column -> PV's PSUM row 64 accumulates the
    softmax denominators for free; normalization is applied to the small
    attnT [64, q] result (reciprocal + gpsimd partition_broadcast + DVE mul);
  - attnT [hd, q] is exactly the lhsT the o_proj matmul needs.

Schedule ("ride" software pipeline): the exp of head r's scores is the ACT
bottleneck, so while head r's scores stream, the PE interleaves the PV
matmuls of head r-1 (whose exp tiles all exist) plus filler work pulled
from a generator queue (remaining QKV projection chunks, then o_proj of
the previous head group). PSUM: 2x[128,1024] score tiles + 1x[65,1024] PV
accumulator + 2x[128,512] projection/o_proj tiles = exactly 8 banks.
"""

import functools
from collections import deque

import numpy as np
import ml_dtypes

import concourse.bacc as bacc
import concourse.bass as bass
import concourse.mybir as mybir
import concourse.tile as tile
from concourse.bass_utils import run_bass_kernel_spmd

B, S, D = 2, 2048, 2048
H, KVH, HD = 32, 8, 64
NCORES = 8
QH = H // NCORES            # 4 q heads per core
ST = B * S                  # 4096 flattened rows
QHD = QH * HD               # 256 (q hd dims per core)
SCALE = 1.0 / np.sqrt(HD)

BF16 = mybir.dt.bfloat16
F32 = mybir.dt.float32

DC = D // 128               # 16 contraction chunks
SC_N = ST // 512            # 8 s-chunks for projections
KB_N = S // 128             # 16 key blocks per batch
QHALF = 1024                # q columns per attention job


def build_program(trace_friendly: bool = False):
    nc = bacc.Bacc("TRN2", target_bir_lowering=False)
    ht = nc.dram_tensor("ht", [D, ST], BF16, kind="ExternalInput")
    wq = nc.dram_tensor("wq", [D, QHD], BF16, kind="ExternalInput")
    wk = nc.dram_tensor("wk", [D, HD], BF16, kind="ExternalInput")
    wv = nc.dram_tensor("wv", [D, HD], BF16, kind="ExternalInput")
    wo = nc.dram_tensor("wo", [QHD, D], BF16, kind="ExternalInput")
    out = nc.dram_tensor("out", [ST, D], BF16, kind="ExternalOutput")

    with tile.TileContext(nc) as tc:
        with (
            tc.tile_pool(name="singles", bufs=1) as singles,
            tc.tile_pool(name="hstream", bufs=2) as hstream,
            tc.tile_pool(name="expp", bufs=26) as expp,
            tc.tile_pool(name="attn", bufs=2) as attnp,
            tc.tile_pool(name="norm", bufs=2) as normp,
            tc.tile_pool(name="ostage", bufs=4) as ostage,
            tc.tile_pool(name="vtp", bufs=2) as vtp,
            tc.tile_pool(name="ps_sc", bufs=2, space="PSUM") as ps_sc,
            tc.tile_pool(name="ps_out", bufs=1, space="PSUM") as ps_out,
            tc.tile_pool(name="ps_op", bufs=2, space="PSUM") as ps_op,
        ):
            # ---- resident weights (one batched DMA per tensor) ----
            # wkv packs K | V columns so one matmul computes both projections:
            # out rows 0:64 = K^T chunk, rows 64:128 = V^T chunk.
            wq_sb = singles.tile([128, DC, QHD], BF16)
            wkv_sb = singles.tile([128, DC, 2 * HD], BF16)
            wo_sb = singles.tile([128, 2, D], BF16)
            # first h-chunk load goes out before the weights: the first Q
            # matmul needs both, and the h transfer is the long pole
            h_sb0 = hstream.tile([128, DC, 512], BF16, tag="h_sb",
                                 name="h_sb0")
            # split the first h-chunk and wq loads so the very first Q
            # matmuls (low dc) can start as soon as possible: eighth-sized
            # leading pieces, then quarters
            splits = [(0, 4), (4, 8), (8, 12), (12, 16)]
            for d0, d1 in splits:
                r0, r1 = d0 * 128, d1 * 128
                nc.sync.dma_start(
                    wq_sb[:, d0:d1],
                    wq[r0:r1, :].rearrange("(dc p) n -> p dc n", p=128))
                nc.sync.dma_start(
                    h_sb0[:, d0:d1],
                    ht[r0:r1, 0:512].rearrange("(dc p) n -> p dc n", p=128))
            nc.sync.dma_start(
                wkv_sb[:, :, 0:HD],
                wk[:, :].rearrange("(dc p) n -> p dc n", p=128))
            nc.sync.dma_start(
                wkv_sb[:, :, HD:2 * HD],
                wv[:, :].rearrange("(dc p) n -> p dc n", p=128))
            ident = singles.tile([64, 64], BF16)
            from concourse.masks import make_identity
            make_identity(nc, ident)

            # ---- resident activations (per batch) ----
            qt_sb = [[singles.tile([64, S], BF16, tag=f"qt{h}_{b}",
                                   name=f"qt{h}_{b}")
                      for b in range(B)] for h in range(QH)]
            kt_sb = [singles.tile([64, S], BF16, tag=f"kt{b}", name=f"kt{b}")
                     for b in range(B)]
            vaug_sb = [singles.tile([128, KB_N, HD + 1], BF16, tag=f"vaug{b}",
                                    name=f"vaug{b}")
                       for b in range(B)]
            for b in range(B):
                nc.vector.memset(vaug_sb[b][:, :, HD:HD + 1], 1.0)

            # ================= phase 1: QKV projections =================
            # b-major s-chunks so batch-0 attention can start early. The
            # last chunk is deferred into the ride stream (ride 0 has no PV
            # work yet, so the PE slack there absorbs it).
            def dma_h(sc):
                h_sb = hstream.tile([128, DC, 512], BF16, tag="h_sb",
                                    name=f"h_sb{sc}")
                nc.sync.dma_start(
                    h_sb[:],
                    ht[:, sc * 512:(sc + 1) * 512].rearrange(
                        "(dc p) n -> p dc n", p=128))
                return h_sb

            def kv_work(sc, h_sb):
                b = sc // (SC_N // B)
                scol = (sc % (SC_N // B)) * 512  # column offset within batch
                # K^T and V^T in one matmul (wkv stationary packs both)
                pkv = ps_op.tile([128, 512], F32, tag="po")
                for dc in range(DC):
                    nc.tensor.matmul(pkv, wkv_sb[:, dc], h_sb[:, dc],
                                     start=(dc == 0), stop=(dc == DC - 1))
                    yield
                nc.vector.tensor_copy(kt_sb[b][:, scol:scol + 512], pkv[0:64])
                vt = vtp.tile([64, 512], BF16)
                nc.vector.tensor_copy(vt, pkv[64:128])
                # transpose V^T [64, s] chunks -> vaug [s, 64] via PE
                for sb in range(4):
                    kb = (scol // 512) * 4 + sb
                    pvt = ps_op.tile([128, 512], F32, tag="po")
                    pvt_bf = pvt.bitcast(BF16)[:, 0:HD]
                    nc.tensor.transpose(
                        pvt_bf, vt[:, sb * 128:(sb + 1) * 128], ident)
                    yield
                    nc.vector.tensor_copy(vaug_sb[b][:, kb, 0:HD], pvt_bf)

            def q_work(sc, h_sb):
                b = sc // (SC_N // B)
                scol = (sc % (SC_N // B)) * 512
                # Q^T: two 128-row chunks of hd
                for m in range(2):
                    pq = ps_op.tile([128, 512], F32, tag="po")
                    for dc in range(DC):
                        nc.tensor.matmul(
                            pq,
                            wq_sb[:, dc, m * 128:(m + 1) * 128],
                            h_sb[:, dc],
                            start=(dc == 0), stop=(dc == DC - 1))
                        yield
                    for hs in range(2):
                        h = 2 * m + hs
                        nc.vector.tensor_copy(
                            qt_sb[h][b][:, scol:scol + 512],
                            pq[hs * 64:(hs + 1) * 64, :])

            def full_work(sc, dma_next=None):
                # body runs lazily at first next(): h_tiles[sc] exists by
                # then (created by the predecessor generator's dma_next)
                if dma_next is not None:
                    h_tiles[dma_next] = dma_h(dma_next)
                h_sb = h_tiles[sc]
                yield from kv_work(sc, h_sb)
                yield from q_work(sc, h_sb)

            # inline prefix: everything ride 0/1 depends on (kt/vaug of
            # batch 0 and the first q-half's qt); the rest streams through
            # the ride filler queue. Sections are round-robined so one
            # section's transposes hide in another's accumulation matmuls.
            def drive_rr(gens):
                gens = deque(gens)
                while gens:
                    g = gens.popleft()
                    try:
                        next(g)
                        next(g)
                        gens.append(g)
                    except StopIteration:
                        pass

            h_tiles = {0: h_sb0}
            for sc in (1, 2, 3):
                h_tiles[sc] = dma_h(sc)
            # o_proj weights only matter from the first o_proj ride (~40%
            # in): load them after the h-chunk stream
            nc.sync.dma_start(
                wo_sb[:], wo[:, :].rearrange("(hh p) n -> p hh n", p=128))
            # NOTE: at most TWO sections per round-robin — each section can
            # hold one "po" PSUM buffer mid-accumulation and the pool only
            # has two; a trio deadlocks the in-order PE queue.
            drive_rr([q_work(0, h_tiles[0])])
            drive_rr([kv_work(0, h_tiles[0]), q_work(1, h_tiles[1])])
            h_tiles[4] = dma_h(4)
            h_tiles[5] = dma_h(5)
            # kv_work(2)/(3) ride along ride 0 (front-loaded fills there):
            # ride-0 scores need their kt copies only from iters 8/12, and
            # ride-1 PV needs their vaug transposes from iter ~6
            proj_gens = [
                kv_work(1, h_tiles[1]),
                kv_work(2, h_tiles[2]),
                kv_work(3, h_tiles[3]),
                q_work(2, h_tiles[2]),
                q_work(3, h_tiles[3]),
                full_work(4, dma_next=6),
                full_work(5, dma_next=7),
                full_work(6),
                full_work(7),
            ]

            # ============ phase 2: attention + o_proj ============
            # Ride-along software pipeline: during ride r the PE streams
            # scores+exp for head r while the PV matmuls of head r-1 (whose
            # exp tiles all exist) and o_proj matmuls of the previous head
            # GROUP interleave into the exp-wait bubbles. PV start is
            # delayed PV_DELAY iterations so the norm of head r-2 (same
            # PSUM accumulator, bufs=1) has drained.


            def oproj_gen(attn_t, b_, q0_, act_copies=False, qcs=None):
                for qc in qcs if qcs is not None else range(QHALF // 128):
                    osb = ostage.tile([128, D], BF16)
                    for nb in range(D // 512):
                        if act_copies and (qc * 4 + nb) % 2 == 0:
                            # tail: scores are finished, so their PSUM banks
                            # double the po rotation depth (copy latency no
                            # longer gates the matmul stream)
                            po = ps_sc.tile([128, 512], F32, tag="scp",
                                            name=f"po_t{qc}_{nb}")
                        else:
                            po = ps_op.tile([128, 512], F32, tag="po")
                        for hh in range(2):
                            nc.tensor.matmul(
                                po, attn_t[:, hh, qc * 128:(qc + 1) * 128],
                                wo_sb[:, hh, nb * 512:(nb + 1) * 512],
                                start=(hh == 0), stop=(hh == 1))
                            yield
                        if act_copies and nb % 2 == 0:
                            # tail has no exp work: ACT takes half the copies
                            nc.scalar.activation(
                                osb[:, nb * 512:(nb + 1) * 512], po,
                                mybir.ActivationFunctionType.Identity)
                        else:
                            nc.vector.tensor_copy(
                                osb[:, nb * 512:(nb + 1) * 512], po)
                    row = b_ * S + q0_ + qc * 128
                    if act_copies:
                        # tail: split the stores so the final DMA (which
                        # gates the drain) starts and finishes sooner
                        nsplit = 2
                        w = D // nsplit
                        for s in range(nsplit):
                            nc.sync.dma_start(
                                out[row:row + 128, s * w:(s + 1) * w],
                                osb[:, s * w:(s + 1) * w])
                    else:
                        nc.sync.dma_start(out[row:row + 128, :], osb)

            ogen = deque(proj_gens)
            pending_ogen = deque()   # normed groups waiting for entry iter

            def fill(n):
                k = 0
                while k < n and ogen:
                    try:
                        next(ogen[0])
                        k += 1
                    except StopIteration:
                        ogen.popleft()
                return k

            rides = [(b, qh, h)
                     for b in range(B) for qh in range(S // QHALF)
                     for h in range(QH)]
            # PV of ride r-1 spans iters 2..12 of ride r; norm fires mid-ride
            # right after the last PV unit so by the NEXT ride's start the
            # outp accumulator (WAR) and the attn tile (RAW for o_proj) are
            # both clear.
            # per-iter PE feed (2 scores + pv + fills) tracks the ACT rate:
            # front iters 2sc+3pv = 1278ns, tail iters 2sc+3op = 1065ns vs
            # exp at 1038ns. PV ends iter 10 so the norm chain drains before
            # the NEXT ride's PV start=True hits the same accumulator.
            pv_sched = [4] * 8 + [0] * 8
            assert sum(pv_sched) == 2 * KB_N and len(pv_sched) == KB_N
            # keep the PE fed ~6 matmuls/iter: fillers top up whatever the
            # scores+PV stream leaves
            fill_sched = [max(0, 6 - 2 - npv) for npv in pv_sched]
            OPROJ_ENTRY_ITER = 13   # lets the norm mul finish before o_proj
            # the flush ride has no scores to pace against: PV all-out so the
            # last norm (and the last group's o_proj) starts ASAP
            pv_sched_flush = [16] * 2 + [0] * 14
            pv_sched_last = [8, 8, 4, 4, 4, 4] + [0] * 10

            attn_by_group = {}

            def norm_head(pb, pqh, ph, outp, c0=0, c1=QHALF):
                gkey = (pb, pqh)
                if gkey not in attn_by_group:
                    attn_by_group[gkey] = attnp.tile(
                        [128, 2, QHALF], BF16, tag="attn",
                        name=f"attn_{pb}_{pqh}")
                attn_t = attn_by_group[gkey]
                # one f32 copy releases the PSUM accumulator ~3us earlier
                # than letting the recip->bcast->mul chain read it directly
                nsb = normp.tile([HD + 1, QHALF], F32, tag="nsb")
                nc.vector.tensor_copy(nsb[:, c0:c1], outp[:, c0:c1])
                recip = normp.tile([1, QHALF], F32, tag="recip")
                nc.vector.reciprocal(recip[:, c0:c1], nsb[HD:HD + 1, c0:c1])
                bcast = normp.tile([64, QHALF], F32, tag="bcast")
                nc.gpsimd.partition_broadcast(
                    bcast[:, c0:c1], recip[:, c0:c1])
                nc.vector.tensor_mul(
                    attn_t[(ph % 2) * 64:(ph % 2) * 64 + 64, ph // 2, c0:c1],
                    nsb[0:HD, c0:c1], bcast[:, c0:c1])
                if ph == QH - 1 and c1 == QHALF:
                    last = (pb, pqh) == (B - 1, S // QHALF - 1)
                    pending_ogen.append(
                        oproj_gen(attn_t, pb, pqh * QHALF, act_copies=last))
                    del attn_by_group[gkey]

            prev = prev_exp = None
            for r in range(len(rides) + 1):
                cur = rides[r] if r < len(rides) else None
                if prev is not None:
                    pb, pqh, ph = prev
                    outp = ps_out.tile([HD + 1, QHALF], F32)
                    if cur is None:
                        # flush: qq-major so each half's norm fires as soon
                        # as that half's accumulation completes
                        pv_units = [(qq, kb)
                                    for qq in range(2) for kb in range(KB_N)]
                    else:
                        pv_units = [(qq, kb)
                                    for kb in range(KB_N) for qq in range(2)]
                cur_exp = {}
                for it in range(KB_N):
                    if cur is not None:
                        cb, cqh, ch = cur
                        q0 = cqh * QHALF
                        scp = ps_sc.tile([128, QHALF], F32)
                        for qq in range(2):
                            nc.tensor.matmul(
                                scp[:, qq * 512:(qq + 1) * 512],
                                kt_sb[cb][:, it * 128:(it + 1) * 128],
                                qt_sb[ch][cb][:, q0 + qq * 512:q0 + (qq + 1) * 512],
                                start=True, stop=True)
                        expT = expp.tile([128, QHALF], BF16)
                        nc.scalar.activation(
                            expT, scp, mybir.ActivationFunctionType.Exp,
                            scale=SCALE)
                        cur_exp[it] = expT
                    sched = pv_sched if cur is not None else pv_sched_flush
                    npv = sched[it] if prev is not None else 0
                    for _ in range(npv):
                        qq, kb = pv_units.pop(0)
                        nc.tensor.matmul(
                            outp[:, qq * 512:(qq + 1) * 512],
                            vaug_sb[pb][:, kb],
                            prev_exp[kb][:, qq * 512:(qq + 1) * 512],
                            start=(kb == 0), stop=(kb == KB_N - 1),
                            skip_group_check=True)
                        if cur is None and len(pv_units) == KB_N:
                            norm_head(pb, pqh, ph, outp, 0, 512)
                    if prev is not None and not pv_units and outp is not None:
                        if cur is None:
                            norm_head(pb, pqh, ph, outp, 512, QHALF)
                        else:
                            norm_head(pb, pqh, ph, outp)
                        outp = None
                    if it >= OPROJ_ENTRY_ITER or cur is None:
                        while pending_ogen:
                            ogen.append(pending_ogen.popleft())
                    if prev is None:
                        # ride 0: front-load so kv_work(2)/(3) beat the
                        # iter-8/12 kt deadlines of this ride's own scores
                        nfill = 5
                    elif cur is None:
                        nfill = 3
                    else:
                        nfill = fill_sched[it]
                    fill(nfill)
                prev, prev_exp = cur, cur_exp
            while pending_ogen:
                ogen.append(pending_ogen.popleft())
            while ogen:
                fill(1 << 20)
    nc.compile()
    return nc


@functools.lru_cache(maxsize=1)
def _get_program():
    return build_program()


def kernel(hidden_states, Wq, Wk, Wv, Wo):
    hidden_states = np.asarray(hidden_states)
    Wq, Wk, Wv, Wo = (np.asarray(x) for x in (Wq, Wk, Wv, Wo))
    bf = ml_dtypes.bfloat16

    htT = np.ascontiguousarray(
        hidden_states.reshape(ST, D).T.astype(bf))          # [D, B*S]
    in_maps = []
    for c in range(NCORES):
        in_maps.append({
            "ht": htT,
            "wq": np.ascontiguousarray(Wq[:, c * QHD:(c + 1) * QHD].astype(bf)),
            "wk": np.ascontiguousarray(Wk[:, c * HD:(c + 1) * HD].astype(bf)),
            "wv": np.ascontiguousarray(Wv[:, c * HD:(c + 1) * HD].astype(bf)),
            "wo": np.ascontiguousarray(Wo[c * QHD:(c + 1) * QHD, :].astype(bf)),
        })

    nc = _get_program()
    res = run_bass_kernel_spmd(nc, in_maps, core_ids=list(range(NCORES)))
    total = res.results[0]["out"].astype(np.float64)
    for c in range(1, NCORES):
        total += res.results[c]["out"].astype(np.float64)
    return total.reshape(B, S, D).astype(np.float32)



# revision 85
# speedup vs baseline: 1.0505x; 1.0041x over previous
"""GQA attention (B=2, S=2048, D=2048, 32 Q heads / 8 KV heads, HD=64) on 8 trn2 cores.

Sharding: tensor-parallel over heads. Core c gets Q heads [4c, 4c+4), KV head c.
Each core computes a full [B*S, D] partial of the output (its 4 heads through
o_proj); the host sums the 8 bf16 partials in f64. No collectives.

On-chip layout:
  - host passes hidden^T [D, B*S] (bf16) so every projection matmul has the
    contraction dim on partitions without any on-chip transpose;
  - K and V projections are fused into one matmul (wkv packs both weight
    blocks); V^T is then flipped to [keys, hd] with tiny PE transposes;
  - scores are computed transposed (scoresT[k, q] = K^T-stationary @ Q^T-moving)
    so the PV matmul consumes exp(scoresT) directly as the moving operand;
  - V is augmented with a ones column -> PV's PSUM row 64 accumulates the
    softmax denominators for free; normalization is applied to the small
    attnT [64, q] result (reciprocal + gpsimd partition_broadcast + DVE mul);
  - attnT [hd, q] is exactly the lhsT the o_proj matmul needs.

Schedule ("ride" software pipeline): the exp of head r's scores is the ACT
bottleneck, so while head r's scores stream, the PE interleaves the PV
matmuls of head r-1 (whose exp tiles all exist) plus filler work pulled
from a generator queue (remaining QKV projection chunks, then o_proj of
the previous head group). PSUM: 2x[128,1024] score tiles + 1x[65,1024] PV
accumulator + 2x[128,512] projection/o_proj tiles = exactly 8 banks.
"""

import functools
from collections import deque

import numpy as np
import ml_dtypes

import concourse.bacc as bacc
import concourse.bass as bass
import concourse.mybir as mybir
import concourse.tile as tile
from concourse.bass_utils import run_bass_kernel_spmd

B, S, D = 2, 2048, 2048
H, KVH, HD = 32, 8, 64
NCORES = 8
QH = H // NCORES            # 4 q heads per core
ST = B * S                  # 4096 flattened rows
QHD = QH * HD               # 256 (q hd dims per core)
SCALE = 1.0 / np.sqrt(HD)

BF16 = mybir.dt.bfloat16
F32 = mybir.dt.float32

DC = D // 128               # 16 contraction chunks
SC_N = ST // 512            # 8 s-chunks for projections
KB_N = S // 128             # 16 key blocks per batch
QHALF = 1024                # q columns per attention job


def build_program(trace_friendly: bool = False):
    nc = bacc.Bacc("TRN2", target_bir_lowering=False)
    ht = nc.dram_tensor("ht", [D, ST], BF16, kind="ExternalInput")
    wq = nc.dram_tensor("wq", [D, QHD], BF16, kind="ExternalInput")
    wk = nc.dram_tensor("wk", [D, HD], BF16, kind="ExternalInput")
    wv = nc.dram_tensor("wv", [D, HD], BF16, kind="ExternalInput")
    wo = nc.dram_tensor("wo", [QHD, D], BF16, kind="ExternalInput")
    out = nc.dram_tensor("out", [ST, D], BF16, kind="ExternalOutput")

    with tile.TileContext(nc) as tc:
        with (
            tc.tile_pool(name="singles", bufs=1) as singles,
            tc.tile_pool(name="hstream", bufs=2) as hstream,
            tc.tile_pool(name="expp", bufs=26) as expp,
            tc.tile_pool(name="attn", bufs=2) as attnp,
            tc.tile_pool(name="norm", bufs=2) as normp,
            tc.tile_pool(name="ostage", bufs=4) as ostage,
            tc.tile_pool(name="vtp", bufs=2) as vtp,
            tc.tile_pool(name="ps_sc", bufs=2, space="PSUM") as ps_sc,
            tc.tile_pool(name="ps_out", bufs=1, space="PSUM") as ps_out,
            tc.tile_pool(name="ps_op", bufs=2, space="PSUM") as ps_op,
        ):
            # ---- resident weights (one batched DMA per tensor) ----
            # wkv packs K | V columns so one matmul computes both projections:
            # out rows 0:64 = K^T chunk, rows 64:128 = V^T chunk.
            wq_sb = singles.tile([128, DC, QHD], BF16)
            wkv_sb = singles.tile([128, DC, 2 * HD], BF16)
            wo_sb = singles.tile([128, 2, D], BF16)
            # first h-chunk load goes out before the weights: the first Q
            # matmul needs both, and the h transfer is the long pole
            h_sb0 = hstream.tile([128, DC, 512], BF16, tag="h_sb",
                                 name="h_sb0")
            # split the first h-chunk and wq loads so the very first Q
            # matmuls (low dc) can start as soon as possible: eighth-sized
            # leading pieces, then quarters
            splits = [(0, 4), (4, 8), (8, 12), (12, 16)]
            for d0, d1 in splits:
                r0, r1 = d0 * 128, d1 * 128
                nc.sync.dma_start(
                    wq_sb[:, d0:d1],
                    wq[r0:r1, :].rearrange("(dc p) n -> p dc n", p=128))
                nc.sync.dma_start(
                    h_sb0[:, d0:d1],
                    ht[r0:r1, 0:512].rearrange("(dc p) n -> p dc n", p=128))
            nc.sync.dma_start(
                wkv_sb[:, :, 0:HD],
                wk[:, :].rearrange("(dc p) n -> p dc n", p=128))
            nc.sync.dma_start(
                wkv_sb[:, :, HD:2 * HD],
                wv[:, :].rearrange("(dc p) n -> p dc n", p=128))
            ident = singles.tile([64, 64], BF16)
            from concourse.masks import make_identity
            make_identity(nc, ident)

            # ---- resident activations (per batch) ----
            qt_sb = [[singles.tile([64, S], BF16, tag=f"qt{h}_{b}",
                                   name=f"qt{h}_{b}")
                      for b in range(B)] for h in range(QH)]
            kt_sb = [singles.tile([64, S], BF16, tag=f"kt{b}", name=f"kt{b}")
                     for b in range(B)]
            vaug_sb = [singles.tile([128, KB_N, HD + 1], BF16, tag=f"vaug{b}",
                                    name=f"vaug{b}")
                       for b in range(B)]
            for b in range(B):
                nc.vector.memset(vaug_sb[b][:, :, HD:HD + 1], 1.0)

            # ================= phase 1: QKV projections =================
            # b-major s-chunks so batch-0 attention can start early. The
            # last chunk is deferred into the ride stream (ride 0 has no PV
            # work yet, so the PE slack there absorbs it).
            def dma_h(sc):
                h_sb = hstream.tile([128, DC, 512], BF16, tag="h_sb",
                                    name=f"h_sb{sc}")
                nc.sync.dma_start(
                    h_sb[:],
                    ht[:, sc * 512:(sc + 1) * 512].rearrange(
                        "(dc p) n -> p dc n", p=128))
                return h_sb

            def kv_work(sc, h_sb):
                b = sc // (SC_N // B)
                scol = (sc % (SC_N // B)) * 512  # column offset within batch
                # K^T and V^T in one matmul (wkv stationary packs both)
                pkv = ps_op.tile([128, 512], F32, tag="po")
                for dc in range(DC):
                    nc.tensor.matmul(pkv, wkv_sb[:, dc], h_sb[:, dc],
                                     start=(dc == 0), stop=(dc == DC - 1))
                    yield
                nc.vector.tensor_copy(kt_sb[b][:, scol:scol + 512], pkv[0:64])
                vt = vtp.tile([64, 512], BF16)
                nc.vector.tensor_copy(vt, pkv[64:128])
                # transpose V^T [64, s] chunks -> vaug [s, 64] via PE
                for sb in range(4):
                    kb = (scol // 512) * 4 + sb
                    pvt = ps_op.tile([128, 512], F32, tag="po")
                    pvt_bf = pvt.bitcast(BF16)[:, 0:HD]
                    nc.tensor.transpose(
                        pvt_bf, vt[:, sb * 128:(sb + 1) * 128], ident)
                    yield
                    nc.vector.tensor_copy(vaug_sb[b][:, kb, 0:HD], pvt_bf)

            def q_work(sc, h_sb):
                b = sc // (SC_N // B)
                scol = (sc % (SC_N // B)) * 512
                # Q^T: two 128-row chunks of hd
                for m in range(2):
                    pq = ps_op.tile([128, 512], F32, tag="po")
                    for dc in range(DC):
                        nc.tensor.matmul(
                            pq,
                            wq_sb[:, dc, m * 128:(m + 1) * 128],
                            h_sb[:, dc],
                            start=(dc == 0), stop=(dc == DC - 1))
                        yield
                    for hs in range(2):
                        h = 2 * m + hs
                        nc.vector.tensor_copy(
                            qt_sb[h][b][:, scol:scol + 512],
                            pq[hs * 64:(hs + 1) * 64, :])

            def full_work(sc, dma_next=None):
                # body runs lazily at first next(): h_tiles[sc] exists by
                # then (created by the predecessor generator's dma_next)
                if dma_next is not None:
                    h_tiles[dma_next] = dma_h(dma_next)
                h_sb = h_tiles[sc]
                yield from kv_work(sc, h_sb)
                yield from q_work(sc, h_sb)

            # inline prefix: everything ride 0/1 depends on (kt/vaug of
            # batch 0 and the first q-half's qt); the rest streams through
            # the ride filler queue. Sections are round-robined so one
            # section's transposes hide in another's accumulation matmuls.
            def drive_rr(gens):
                gens = deque(gens)
                while gens:
                    g = gens.popleft()
                    try:
                        next(g)
                        next(g)
                        gens.append(g)
                    except StopIteration:
                        pass

            h_tiles = {0: h_sb0}
            # h1 in quarters: its consumers (q1/kv1) start on the first
            # quarter instead of waiting out the full 5.8us transfer
            h_sb1 = hstream.tile([128, DC, 512], BF16, tag="h_sb",
                                 name="h_sb1")
            for d0, d1 in ((0, 4), (4, 8), (8, 12), (12, 16)):
                nc.sync.dma_start(
                    h_sb1[:, d0:d1],
                    ht[d0 * 128:d1 * 128, 512:1024].rearrange(
                        "(dc p) n -> p dc n", p=128))
            h_tiles[1] = h_sb1
            for sc in (2, 3):
                h_tiles[sc] = dma_h(sc)
            # o_proj weights only matter from the first o_proj ride (~40%
            # in): load them after the h-chunk stream
            nc.sync.dma_start(
                wo_sb[:], wo[:, :].rearrange("(hh p) n -> p hh n", p=128))
            # NOTE: at most TWO sections per round-robin — each section can
            # hold one "po" PSUM buffer mid-accumulation and the pool only
            # has two; a trio deadlocks the in-order PE queue.
            drive_rr([q_work(0, h_tiles[0])])
            drive_rr([kv_work(0, h_tiles[0]), q_work(1, h_tiles[1])])
            h_tiles[4] = dma_h(4)
            h_tiles[5] = dma_h(5)
            # kv_work(2)/(3) ride along ride 0 (front-loaded fills there):
            # ride-0 scores need their kt copies only from iters 8/12, and
            # ride-1 PV needs their vaug transposes from iter ~6
            proj_gens = [
                kv_work(1, h_tiles[1]),
                kv_work(2, h_tiles[2]),
                kv_work(3, h_tiles[3]),
                q_work(2, h_tiles[2]),
                q_work(3, h_tiles[3]),
                full_work(4, dma_next=6),
                full_work(5, dma_next=7),
                full_work(6),
                full_work(7),
            ]

            # ============ phase 2: attention + o_proj ============
            # Ride-along software pipeline: during ride r the PE streams
            # scores+exp for head r while the PV matmuls of head r-1 (whose
            # exp tiles all exist) and o_proj matmuls of the previous head
            # GROUP interleave into the exp-wait bubbles. PV start is
            # delayed PV_DELAY iterations so the norm of head r-2 (same
            # PSUM accumulator, bufs=1) has drained.


            def oproj_gen(attn_t, b_, q0_, act_copies=False, qcs=None):
                for qc in qcs if qcs is not None else range(QHALF // 128):
                    osb = ostage.tile([128, D], BF16)
                    for nb in range(D // 512):
                        if act_copies and (qc * 4 + nb) % 2 == 0:
                            # tail: scores are finished, so their PSUM banks
                            # double the po rotation depth (copy latency no
                            # longer gates the matmul stream)
                            po = ps_sc.tile([128, 512], F32, tag="scp",
                                            name=f"po_t{qc}_{nb}")
                        else:
                            po = ps_op.tile([128, 512], F32, tag="po")
                        for hh in range(2):
                            nc.tensor.matmul(
                                po, attn_t[:, hh, qc * 128:(qc + 1) * 128],
                                wo_sb[:, hh, nb * 512:(nb + 1) * 512],
                                start=(hh == 0), stop=(hh == 1))
                            yield
                        if act_copies and nb % 2 == 0:
                            # tail has no exp work: ACT takes half the copies
                            nc.scalar.activation(
                                osb[:, nb * 512:(nb + 1) * 512], po,
                                mybir.ActivationFunctionType.Identity)
                        else:
                            nc.vector.tensor_copy(
                                osb[:, nb * 512:(nb + 1) * 512], po)
                    row = b_ * S + q0_ + qc * 128
                    if act_copies:
                        # tail: split the stores so the final DMA (which
                        # gates the drain) starts and finishes sooner
                        nsplit = 2
                        w = D // nsplit
                        for s in range(nsplit):
                            nc.sync.dma_start(
                                out[row:row + 128, s * w:(s + 1) * w],
                                osb[:, s * w:(s + 1) * w])
                    else:
                        nc.sync.dma_start(out[row:row + 128, :], osb)

            ogen = deque(proj_gens)
            pending_ogen = deque()   # normed groups waiting for entry iter

            def fill(n):
                k = 0
                while k < n and ogen:
                    try:
                        next(ogen[0])
                        k += 1
                    except StopIteration:
                        ogen.popleft()
                return k

            rides = [(b, qh, h)
                     for b in range(B) for qh in range(S // QHALF)
                     for h in range(QH)]
            # PV of ride r-1 spans iters 2..12 of ride r; norm fires mid-ride
            # right after the last PV unit so by the NEXT ride's start the
            # outp accumulator (WAR) and the attn tile (RAW for o_proj) are
            # both clear.
            # per-iter PE feed (2 scores + pv + fills) tracks the ACT rate:
            # front iters 2sc+3pv = 1278ns, tail iters 2sc+3op = 1065ns vs
            # exp at 1038ns. PV ends iter 10 so the norm chain drains before
            # the NEXT ride's PV start=True hits the same accumulator.
            pv_sched = [4] * 8 + [0] * 8
            assert sum(pv_sched) == 2 * KB_N and len(pv_sched) == KB_N
            # keep the PE fed ~6 matmuls/iter: fillers top up whatever the
            # scores+PV stream leaves
            fill_sched = [max(0, 6 - 2 - npv) for npv in pv_sched]
            OPROJ_ENTRY_ITER = 13   # lets the norm mul finish before o_proj
            # the flush ride has no scores to pace against: PV all-out so the
            # last norm (and the last group's o_proj) starts ASAP
            pv_sched_flush = [16] * 2 + [0] * 14
            pv_sched_last = [8, 8, 4, 4, 4, 4] + [0] * 10

            attn_by_group = {}

            def norm_head(pb, pqh, ph, outp, c0=0, c1=QHALF):
                gkey = (pb, pqh)
                if gkey not in attn_by_group:
                    attn_by_group[gkey] = attnp.tile(
                        [128, 2, QHALF], BF16, tag="attn",
                        name=f"attn_{pb}_{pqh}")
                attn_t = attn_by_group[gkey]
                # one f32 copy releases the PSUM accumulator ~3us earlier
                # than letting the recip->bcast->mul chain read it directly
                nsb = normp.tile([HD + 1, QHALF], F32, tag="nsb")
                nc.vector.tensor_copy(nsb[:, c0:c1], outp[:, c0:c1])
                recip = normp.tile([1, QHALF], F32, tag="recip")
                nc.vector.reciprocal(recip[:, c0:c1], nsb[HD:HD + 1, c0:c1])
                bcast = normp.tile([64, QHALF], F32, tag="bcast")
                nc.gpsimd.partition_broadcast(
                    bcast[:, c0:c1], recip[:, c0:c1])
                nc.vector.tensor_mul(
                    attn_t[(ph % 2) * 64:(ph % 2) * 64 + 64, ph // 2, c0:c1],
                    nsb[0:HD, c0:c1], bcast[:, c0:c1])
                if ph == QH - 1 and c1 == QHALF:
                    last = (pb, pqh) == (B - 1, S // QHALF - 1)
                    pending_ogen.append(
                        oproj_gen(attn_t, pb, pqh * QHALF, act_copies=last))
                    del attn_by_group[gkey]

            prev = prev_exp = None
            for r in range(len(rides) + 1):
                cur = rides[r] if r < len(rides) else None
                if prev is not None:
                    pb, pqh, ph = prev
                    outp = ps_out.tile([HD + 1, QHALF], F32)
                    if cur is None:
                        # flush: qq-major so each half's norm fires as soon
                        # as that half's accumulation completes
                        pv_units = [(qq, kb)
                                    for qq in range(2) for kb in range(KB_N)]
                    else:
                        pv_units = [(qq, kb)
                                    for kb in range(KB_N) for qq in range(2)]
                cur_exp = {}
                for it in range(KB_N):
                    if cur is not None:
                        cb, cqh, ch = cur
                        q0 = cqh * QHALF
                        scp = ps_sc.tile([128, QHALF], F32)
                        for qq in range(2):
                            nc.tensor.matmul(
                                scp[:, qq * 512:(qq + 1) * 512],
                                kt_sb[cb][:, it * 128:(it + 1) * 128],
                                qt_sb[ch][cb][:, q0 + qq * 512:q0 + (qq + 1) * 512],
                                start=True, stop=True)
                        expT = expp.tile([128, QHALF], BF16)
                        nc.scalar.activation(
                            expT, scp, mybir.ActivationFunctionType.Exp,
                            scale=SCALE)
                        cur_exp[it] = expT
                    sched = pv_sched if cur is not None else pv_sched_flush
                    npv = sched[it] if prev is not None else 0
                    for _ in range(npv):
                        qq, kb = pv_units.pop(0)
                        nc.tensor.matmul(
                            outp[:, qq * 512:(qq + 1) * 512],
                            vaug_sb[pb][:, kb],
                            prev_exp[kb][:, qq * 512:(qq + 1) * 512],
                            start=(kb == 0), stop=(kb == KB_N - 1),
                            skip_group_check=True)
                        if cur is None and len(pv_units) == KB_N:
                            norm_head(pb, pqh, ph, outp, 0, 512)
                    if prev is not None and not pv_units and outp is not None:
                        if cur is None:
                            norm_head(pb, pqh, ph, outp, 512, QHALF)
                        else:
                            norm_head(pb, pqh, ph, outp)
                        outp = None
                    if it >= OPROJ_ENTRY_ITER or cur is None:
                        while pending_ogen:
                            ogen.append(pending_ogen.popleft())
                    if prev is None:
                        # ride 0: front-load so kv_work(2)/(3) beat the
                        # iter-8/12 kt deadlines of this ride's own scores
                        nfill = 5
                    elif cur is None:
                        nfill = 3
                    else:
                        nfill = fill_sched[it]
                    fill(nfill)
                prev, prev_exp = cur, cur_exp
            while pending_ogen:
                ogen.append(pending_ogen.popleft())
            while ogen:
                fill(1 << 20)
    nc.compile()
    return nc


@functools.lru_cache(maxsize=1)
def _get_program():
    return build_program()


def kernel(hidden_states, Wq, Wk, Wv, Wo):
    hidden_states = np.asarray(hidden_states)
    Wq, Wk, Wv, Wo = (np.asarray(x) for x in (Wq, Wk, Wv, Wo))
    bf = ml_dtypes.bfloat16

    htT = np.ascontiguousarray(
        hidden_states.reshape(ST, D).T.astype(bf))          # [D, B*S]
    in_maps = []
    for c in range(NCORES):
        in_maps.append({
            "ht": htT,
            "wq": np.ascontiguousarray(Wq[:, c * QHD:(c + 1) * QHD].astype(bf)),
            "wk": np.ascontiguousarray(Wk[:, c * HD:(c + 1) * HD].astype(bf)),
            "wv": np.ascontiguousarray(Wv[:, c * HD:(c + 1) * HD].astype(bf)),
            "wo": np.ascontiguousarray(Wo[c * QHD:(c + 1) * QHD, :].astype(bf)),
        })

    nc = _get_program()
    res = run_bass_kernel_spmd(nc, in_maps, core_ids=list(range(NCORES)))
    total = res.results[0]["out"].astype(np.float64)
    for c in range(1, NCORES):
        total += res.results[c]["out"].astype(np.float64)
    return total.reshape(B, S, D).astype(np.float32)

